# revision 39
# baseline (speedup 1.0000x reference)
"""Trainium2 Bass kernel for nn_Attention_interaction (dense_transformer).

Math (per batch b, head h):
    q = l2norm(x);  S = (q @ q^T) / SCALE / attn_gamma;  P = softmax(S, -1)
    o = P @ y;  o2 = o @ W^T + bias;  out = w0*y + w1*o2
with w_i = exp(sum_gamma_i) / (exp(sum_gamma0) + exp(sum_gamma1)).

Sharding: batch dim B=8 across the 8 cores (1 batch x 8 heads per core).
Heads run in 4 pairs; the two heads' S matmuls use disjoint PE row halves
(tile_position) and are emitted adjacently so their rhs streams co-issue.

The kernel is exp/PE-bound (8.4M softmax exps per core; the PE spends
most of the run HAM-throttled at 1.2 GHz), so exp is SPLIT: ACT computes
exact exp for head A (whole [128,1024] chunks, low per-instr overhead)
plus head B's leftover chunks; the DVE computes DVE_CHUNKS of head B's
chunks via a Schraudolph bit-trick (fp8e4 bits = round(A8*s+B8) written
as uint8; softmax normalization cancels most of the error), per
[128,512] half so the S-psum WAR releases at half-chunk granularity.
E is fp8e4 throughout, feeding the DoubleRow O matmuls.  Layout:
  - host prep: q = l2norm(x)*sqrt(c2) pre-transposed per pair into
    [128, N] (head A rows 0-63, B 64-127); fp8 [y | 1 | pad] for the O
    weights; bf16 w0*y + w1*bias for the epilogue add.
  - PSUM (all 8 banks): head B S tiles 2x[128,512] (1 bank each,
    half-granular exp WAR), head A S tile [128,1024] (2 banks), O
    accumulators 4x[65..128,512] per (head, jc) (4 banks; the proj
    tiles reuse them via the tag rings).
  - S matmuls: the two heads' same-jc matmuls are adjacent so their
    disjoint-row-half streams co-issue; O accumulates (E @ [y|1])^T via
    fp8 DoubleRow matmuls (K=256: chunk pairs folded through [128,2,*]
    APs); row 64 = softmax denominators via the ones-column, bounced
    through DRAM into per-partition layout (DMA cannot read PSUM, and
    walrus cannot lower a partition-expanding SBUF->SBUF transpose).
  - proj (w1*W^T matmul = the transpose back to token-major) reuses the
    freed O tiles; bias rides in yb; o2 = pj * rinv is a broadcast-AP
    tensor_tensor per jc; the o2+yb add runs on the otherwise-idle
    GPSIMD mid-stream (SBUF-only operands) and on the DVE for the
    latency-critical last pair.
  - O accumulation runs O_DELAY chunks behind exp so the static in-order
    PE program never stalls on exp or on the PSUM tag reuse WAR.
"""

import math
import os

import numpy as np
import ml_dtypes

import concourse.bass as bass
import concourse.bacc as bacc
import concourse.tile as tile
from concourse import mybir
from concourse.bass import broadcast_tensor_aps
from concourse import bass_utils as _bass_utils
from concourse.bass_utils import run_bass_kernel_spmd
from concourse._compat import get_trn_type

# NOTE: walrus's LDWEIGHTS dedup pass (--enable-ldw-opt=true) would remove
# the redundant weight loads between same-lhsT jc pairs, but walrus_driver
# crashes with it enabled (verified) -- that is why bass pins it off.

B, H, N, D = 8, 8, 1024, 64
SCALE = (512 // 8) ** (-0.5)  # 0.125
EPS = 1e-6
NCORES = 8
NB = N // 128
NW = N * NB
F32 = mybir.dt.float32
BF16 = mybir.dt.bfloat16
FP8 = mybir.dt.float8e4
U8 = mybir.dt.uint8
AX = mybir.AxisListType
OP = mybir.AluOpType
ACT = mybir.ActivationFunctionType
PM = mybir.MatmulPerfMode

A8 = 8.0 / math.log(2.0)
B8 = 7.0 * 8  # e4m3 Schraudolph magic (softmax cancels the offset choice)
DVE_CHUNKS = int(os.environ.get("KERNEL_DVE_CHUNKS", "7"))  # head-B chunks 0..DVE_CHUNKS-1 on the DVE, rest on ACT
O_DELAY = 3
WARMUP_MMS = int(os.environ.get("KERNEL_WARMUP_MMS", "0"))
YAP = 80  # padded ya8 row length (DoubleRow needs 16B-aligned Ko step)

LAST_RESULTS = None


def _emit(ctx, tc):
    nc = tc.nc
    qt = nc.dram_tensor("qt", [H // 2, 128, N], BF16, kind="ExternalInput")
    ya8 = nc.dram_tensor("ya8", [H, N, YAP], FP8, kind="ExternalInput")
    ybb = nc.dram_tensor("ybb", [H, N, D], BF16, kind="ExternalInput")
    wt = nc.dram_tensor("wt", [D, D], BF16, kind="ExternalInput")
    out = nc.dram_tensor("out", [H, N, D], BF16, kind="ExternalOutput")
    rscr = nc.dram_tensor("rscr", [2, 2, N], BF16)

    singles = ctx.enter_context(tc.tile_pool(name="singles", bufs=1))
    io = ctx.enter_context(tc.tile_pool(name="io", bufs=2))
    st = ctx.enter_context(tc.tile_pool(name="st", bufs=2))
    work = ctx.enter_context(tc.tile_pool(name="work", bufs=2))
    epool = ctx.enter_context(tc.tile_pool(name="epool", bufs=2))
    qpool = ctx.enter_context(tc.tile_pool(name="qpool", bufs=1))
    # PSUM: 4 banks of per-(head,jc) [128,512] S tiles + 4 banks of
    # per-(head,jc) O accumulators = all 8 banks.  S tiles are 1-bank and
    # exp consumes them per-half, so the WAR for chunk i+1's S matmul
    # releases ~a full chunk earlier than the old whole-chunk scheme.
    ps_s = ctx.enter_context(tc.tile_pool(name="ps_s", bufs=1, space="PSUM"))
    ps_o = ctx.enter_context(tc.tile_pool(name="ps_o", bufs=1, space="PSUM"))

    # HAM pre-warm: dense F=512 matmuls on garbage SBUF data (no DMA dep,
    # so the burst starts at t~0 and spans the ~8us DMA-ring ramp; the PE
    # must look continuously busy for a full 4096-cycle window to reach
    # K=8/8 before the first real S matmul)
    if WARMUP_MMS:
        warm_sb = singles.tile([64, 512], BF16)
        nc.vector.memset(warm_sb, 1.0)
        wps = ps_s.tile([128, 512], F32, tag="psSB0", name="warm")
        for k in range(WARMUP_MMS):
            nc.tensor.matmul(
                wps[0:64, 0:512], lhsT=warm_sb[:, 0:64], rhs=warm_sb,
                start=True, stop=True, tile_position=(0, 0),
                skip_group_check=True,
            )

    qT = [None] * (H // 2)

    def load_qt(p):
        # two half-loads: the first S matmuls (jc0) only need cols 0-511,
        # so pair 0's stream starts one DMA earlier
        q = qpool.tile([128, N], BF16, tag=f"qT{p}", name=f"qT{p}")
        nc.sync.dma_start(out=q[:, 0:512], in_=qt[p][:, 0:512])
        nc.sync.dma_start(out=q[:, 512:1024], in_=qt[p][:, 512:1024])
        qT[p] = q

    # qt pair 0 is the first thing the PE needs; wt is only read by the
    # first pair's tail proj, so its load can trail the qt halves
    load_qt(0)
    wt_sb = singles.tile([D, D], BF16)
    nc.sync.dma_start(out=wt_sb, in_=wt[:, :])
    load_qt(1)

    def bscale(dst, src, sc):
        sc3 = sc.rearrange("p (b u) -> p b u", u=1)
        sc_b, src_b = broadcast_tensor_aps(sc3, src)
        nc.vector.tensor_tensor(dst, src_b, sc_b, OP.mult)

    def make_state(p):
        hA, hB = 2 * p, 2 * p + 1
        yaA = io.tile([128, NB, YAP], FP8, tag="yaA")
        yaB = io.tile([128, NB, YAP], FP8, tag="yaB")
        ybA = io.tile([128, NB, D], BF16, tag="ybA")
        ybB = io.tile([128, NB, D], BF16, tag="ybB")
        nc.sync.dma_start(out=yaA, in_=ya8[hA].rearrange("(b p) d -> p b d", p=128))
        nc.sync.dma_start(out=yaB, in_=ya8[hB].rearrange("(b p) d -> p b d", p=128))
        EA = epool.tile([128, NW], FP8, tag="EA")
        EB = epool.tile([128, NW], FP8, tag="EB")
        return {
            "p": p, "q": qT[p], "hA": hA, "hB": hB,
            "heads": ((64, EB, yaB, ybB, "B"), (0, EA, yaA, ybA, "A")),
            "okptr": [0, 0], "odone": [0, 0], "otile": [None, None],
        }

    def emit_loads(P):
        # yb is only read in the epilogue, so its loads are issued AFTER
        # the previous pair's latency-critical denominator-bounce DMAs
        hA, hB = P["hA"], P["hB"]
        (_, _, _, ybB, _), (_, _, _, ybA, _) = P["heads"]
        nc.sync.dma_start(out=ybA, in_=ybb[hA].rearrange("(b p) d -> p b d", p=128))
        nc.sync.dma_start(out=ybB, in_=ybb[hB].rearrange("(b p) d -> p b d", p=128))

    def emit_o(P, hidx, flush=False):
        base, E, ytile, ybt, hc = P["heads"][hidx]
        E3 = E.rearrange("p (i n) -> p i n", n=N)
        while P["okptr"][hidx] < NB // 2:
            k = P["okptr"][hidx]
            # pair 0 has no previous-pair tail work to fill its exp-pipeline
            # ramp (measured 3.7us chunk deltas while the PE starves), so its
            # O k-groups are emitted one chunk earlier; the HOL risk is
            # bounded (~0.4us: O-k0 waits only chunks 0-1's exp halves)
            delay = 2 if P["p"] == 0 else 3
            if not flush and 2 * k + delay > P["odone"][hidx]:
                return
            if k == 0:
                P["otile"][hidx] = [
                    ps_o.tile(
                        [128, 512], F32, tag=f"o{hc}{jc}",
                        name=f"ot{hc}{jc}{P['p']}",
                    )
                    for jc in range(2)
                ]
            for jc in range(2):
                nc.tensor.matmul(
                    P["otile"][hidx][jc][0 : D + 1, :],
                    lhsT=ytile[:, 2 * k : 2 * k + 2, 0 : D + 1],
                    rhs=E3[:, 2 * k : 2 * k + 2, jc * 512 : (jc + 1) * 512],
                    start=(k == 0), stop=(k == NB // 2 - 1),
                    perf_mode=PM.DoubleRow, tile_position=(0, 0),
                )
            P["okptr"][hidx] += 1

    def emit_chunk(P, i):
        q = P["q"]
        # head B: two 1-bank tiles, exp'd per-half on the DVE (cheap extra
        # instr, and the WAR for chunk i+1's B matmul releases at half-chunk
        # granularity).  head A: one 2-bank tile, exp'd whole on ACT (the
        # per-instruction ACTIVATE overhead is ~300 cycles, so halving A's
        # exps would cost ~2.9us/pair on the pacing engine).
        pssB = [
            ps_s.tile([128, 512], F32, tag=f"psSB{jc}", name=f"psSB{jc}")
            for jc in range(2)
        ]
        pssA = ps_s.tile([128, N], F32, tag="psSA", name="psSA")
        for jc in range(2):
            for hidx, (base, E, ytile, ybt, hc) in enumerate(P["heads"]):
                nc.tensor.matmul(
                    pssB[jc] if hc == "B"
                    else pssA[:, jc * 512 : (jc + 1) * 512],
                    lhsT=q[base : base + 64, i * 128 : (i + 1) * 128],
                    rhs=q[base : base + 64, jc * 512 : (jc + 1) * 512],
                    start=True, stop=True, tile_position=(base, 0),
                )
        for hidx, (base, E, ytile, ybt, hc) in enumerate(P["heads"]):
            # B's last chunk also on the DVE so both engines finish the
            # stream in parallel and the tail is not gated by ACT doing
            # both heads' final chunks back-to-back
            on_dve = hc == "B" and (i < DVE_CHUNKS - 1 or i == NB - 1)
            if hc == "B" and on_dve:
                for jc in range(2):
                    d0 = i * N + jc * 512
                    nc.vector.tensor_scalar(
                        out=E.bitcast(U8)[:, d0 : d0 + 512],
                        in0=pssB[jc], scalar1=A8, scalar2=B8,
                        op0=OP.mult, op1=OP.add,
                    )
            elif hc == "B":
                for jc in range(2):
                    d0 = i * N + jc * 512
                    nc.scalar.activation(
                        out=E[:, d0 : d0 + 512], in_=pssB[jc], func=ACT.Exp
                    )
            else:
                nc.scalar.activation(
                    out=E[:, i * N : (i + 1) * N], in_=pssA, func=ACT.Exp
                )
            P["odone"][hidx] += 1
        for hidx in range(2):
            emit_o(P, hidx)

    def emit_tail1(P, last=False):
        """O flush, OT evacuation, denominator bounce, proj.

        Final pair: head A is flushed FIRST (it was the long pole: second
        position + full-evac-gated bounce cost ~5us of exposed DMA latency
        at the very end), both heads evac their denominator row before the
        bulk so the bounce launches right at O-stop, and each head's bounce
        rides its own DMA ring (A=sync, B=scalar) so the two round-trips
        overlap completely."""
        p = P["p"]
        P["rT"] = st.tile([128, 2, NB], BF16, tag="rT", name="rT")
        order = (1, 0) if last else (0, 1)
        for hidx in order:
            base, E, ytile, ybt, hc = P["heads"][hidx]
            emit_o(P, hidx, flush=True)
            OT = work.tile([D + 1, N], BF16, tag=f"OT{hc}")
            if last:
                nc.vector.tensor_copy(
                    OT[D : D + 1, 0:512], P["otile"][hidx][0][D : D + 1, :]
                )
                nc.scalar.copy(
                    OT[D : D + 1, 512:1024], P["otile"][hidx][1][D : D + 1, :]
                )
                deng = nc.sync if hc == "A" else nc.scalar
                deng.dma_start(out=rscr[p % 2, hidx], in_=OT[D : D + 1, :])
                deng.dma_start(
                    out=P["rT"][:, hidx, :],
                    in_=rscr[p % 2, hidx].rearrange("(b p) -> p b", p=128),
                )
                nc.vector.tensor_copy(
                    OT[0:D, 0:512], P["otile"][hidx][0][0:D, :]
                )
                nc.scalar.copy(
                    OT[0:D, 512:1024], P["otile"][hidx][1][0:D, :]
                )
            else:
                # per-jc evac: DVE takes jc0, ACT takes jc1 (moving A-jc0 to
                # ACT to balance engine totals measured WORSE on hardware --
                # the serialized A evacs delay proj-A and the otile-ring WAR
                # more than the DVE relief is worth)
                nc.vector.tensor_copy(
                    OT[:, 0:512], P["otile"][hidx][0][0 : D + 1, :]
                )
                nc.scalar.copy(
                    OT[:, 512:1024], P["otile"][hidx][1][0 : D + 1, :]
                )
                nc.sync.dma_start(out=rscr[p % 2, hidx], in_=OT[D : D + 1, :])
                nc.sync.dma_start(
                    out=P["rT"][:, hidx, :],
                    in_=rscr[p % 2, hidx].rearrange("(b p) -> p b", p=128),
                )
            pj = [
                ps_o.tile(
                    [128, 512], F32, tag=f"o{hc}{jc}", name=f"pj{hc}{jc}{p}"
                )
                for jc in range(2)
            ]
            for b in range(NB):
                nc.tensor.matmul(
                    pj[b // 4][:, (b % 4) * 128 : (b % 4) * 128 + D],
                    lhsT=OT[0:D, b * 128 : (b + 1) * 128],
                    rhs=wt_sb,
                    start=True, stop=True, tile_position=(0, 0),
                )
            P["otile"][hidx] = pj

    def emit_tail2(P, last=False):
        """1/r scale, +yb, store."""
        order = (1, 0) if last else (0, 1)
        for hidx in order:
            base, E, ytile, ybt, hc = P["heads"][hidx]
            ho = P["hB"] if hc == "B" else P["hA"]
            rinv = st.tile([128, NB], F32, tag=f"rinv{hc}")
            nc.vector.reciprocal(rinv, P["rT"][:, hidx, :])
            o2 = work.tile([128, NB, D], BF16, tag=f"o2{hc}", name=f"o2{hc}")
            for jc in range(2):
                pj3 = P["otile"][hidx][jc].rearrange(
                    "p (b c) -> p b c", b=NB // 2
                )[:, :, 0:D]
                bscale(
                    o2[:, 4 * jc : 4 * jc + 4], pj3,
                    rinv[:, 4 * jc : 4 * jc + 4],
                )
            fin = work.tile([128, NB, D], BF16, tag=f"fin{hc}", name=f"fin{hc}")
            # the very last store rides the otherwise-done scalar ring so
            # the two final stores drain in parallel (ACT has no work left,
            # so the queue-DMA-blocks-ACTIVATE hazard doesn't apply here)
            # final pair: each head's store follows its own bounce ring
            # (A=sync, B=scalar) so the two stores drain in parallel
            eng = nc.scalar if last and hc == "B" else nc.sync
            o3 = out[ho].rearrange("(b p) d -> p b d", p=128)
            # fins stay on the DVE: any gpsimd use adds an SWDGE drain to
            # the NEFF postamble (the original design avoided gpsimd for
            # exactly this), and the DVE is not the pacer in the
            # HAM-cold regime anyway
            nc.vector.tensor_tensor(fin, o2, ybt, OP.add)
            eng.dma_start(out=o3, in_=fin)

    # software-pipelined pair schedule: each pair's first two chunk groups
    # are emitted around the previous pair's tail, so ACT/DVE stay fed with
    # exp work while the PE runs the tail's O-flush and proj matmuls and
    # the denominator-bounce DMA latency hides behind the second chunk.
    prev = None
    for p in range(H // 2):
        P = make_state(p)
        if prev is None:
            emit_loads(P)
        emit_chunk(P, 0)
        if prev is not None:
            emit_tail1(prev)
            emit_loads(P)
        emit_chunk(P, 1)
        if prev is not None:
            emit_tail2(prev)
        for i in range(2, NB):
            emit_chunk(P, i)
            if i == 2 and p + 2 < H // 2:
                load_qt(p + 2)
        prev = P
    emit_tail1(prev, last=True)
    emit_tail2(prev, last=True)


def build_program() -> bass.Bass:
    from contextlib import ExitStack

    nc = bacc.Bacc(get_trn_type() or "TRN2", target_bir_lowering=False)
    with tile.TileContext(nc) as tc:
        with ExitStack() as ctx:
            _emit(ctx, tc)
    nc.compile()
    return nc


def kernel(x, y, proj_w, proj_b, attn_gamma, sum_gamma0, sum_gamma1):
    global LAST_RESULTS
    x = np.asarray(x, dtype=np.float32)
    y = np.asarray(y, dtype=np.float32)
    proj_w = np.asarray(proj_w, dtype=np.float32)
    proj_b = np.asarray(proj_b, dtype=np.float32)
    g0 = math.exp(float(np.asarray(sum_gamma0)))
    g1 = math.exp(float(np.asarray(sum_gamma1)))
    w0 = g0 / (g0 + g1)
    w1 = g1 / (g0 + g1)
    c2 = 1.0 / (SCALE * float(np.asarray(attn_gamma)))

    nc = build_program()

    # q = l2norm(x) * sqrt(c2), transposed per pair: [B, 4, 128, N] with
    # head 2p on partitions 0-63 and head 2p+1 on partitions 64-127.
    q = (x * math.sqrt(c2) / np.sqrt((x * x).sum(-1, keepdims=True) + EPS))
    qt = np.ascontiguousarray(
        q.reshape(B, H // 2, 2, N, D).transpose(0, 1, 2, 4, 3).reshape(
            B, H // 2, 128, N
        )
    ).astype(ml_dtypes.bfloat16)
    # fp8 [y | 1 | pad] for the DoubleRow O matmuls; bf16 w0*y + w1*bias
    ya8 = np.zeros(y.shape[:-1] + (YAP,), ml_dtypes.float8_e4m3)
    ya8[..., 0:D] = y.astype(ml_dtypes.float8_e4m3)
    ya8[..., D] = 1.0
    ybb = (w0 * y + w1 * proj_b).astype(ml_dtypes.bfloat16)
    wt = (proj_w.T * w1).astype(ml_dtypes.bfloat16)

    in_maps = [
        {"qt": qt[c], "ya8": ya8[c], "ybb": ybb[c], "wt": wt}
        for c in range(NCORES)
    ]
    res = run_bass_kernel_spmd(nc, in_maps, list(range(NCORES)))
    LAST_RESULTS = res
    return np.stack(
        [res.results[c]["out"].astype(np.float32) for c in range(NCORES)], axis=0
    )



# revision 40
# speedup vs baseline: 1.1569x; 1.1569x over previous
"""Trainium2 Bass kernel for nn_Attention_interaction (dense_transformer).

Math (per batch b, head h):
    q = l2norm(x);  S = (q @ q^T) / SCALE / attn_gamma;  P = softmax(S, -1)
    o = P @ y;  o2 = o @ W^T + bias;  out = w0*y + w1*o2
with w_i = exp(sum_gamma_i) / (exp(sum_gamma0) + exp(sum_gamma1)).

Sharding: batch dim B=8 across the 8 cores (1 batch x 8 heads per core).
Heads run in 4 pairs; the two heads' S matmuls use disjoint PE row halves
(tile_position) and are emitted adjacently so their rhs streams co-issue.

The kernel is exp/PE-bound (8.4M softmax exps per core; the PE spends
most of the run HAM-throttled at 1.2 GHz), so exp is SPLIT: ACT computes
exact exp for head A (whole [128,1024] chunks, low per-instr overhead)
plus head B's leftover chunks; the DVE computes DVE_CHUNKS of head B's
chunks via a Schraudolph bit-trick (fp8e4 bits = round(A8*s+B8) written
as uint8; softmax normalization cancels most of the error), per
[128,512] half so the S-psum WAR releases at half-chunk granularity.
E is fp8e4 throughout, feeding the DoubleRow O matmuls.  Layout:
  - host prep: q = l2norm(x)*sqrt(c2) pre-transposed per pair into
    [128, N] (head A rows 0-63, B 64-127); fp8 [y | 1 | pad] for the O
    weights; bf16 w0*y + w1*bias for the epilogue add.
  - PSUM (all 8 banks): head B S tiles 2x[128,512] (1 bank each,
    half-granular exp WAR), head A S tile [128,1024] (2 banks), O
    accumulators 4x[65..128,512] per (head, jc) (4 banks; the proj
    tiles reuse them via the tag rings).
  - S matmuls: the two heads' same-jc matmuls are adjacent so their
    disjoint-row-half streams co-issue; O accumulates (E @ [y|1])^T via
    fp8 DoubleRow matmuls (K=256: chunk pairs folded through [128,2,*]
    APs); row 64 = softmax denominators via the ones-column, bounced
    through DRAM into per-partition layout (DMA cannot read PSUM, and
    walrus cannot lower a partition-expanding SBUF->SBUF transpose).
  - proj (w1*W^T matmul = the transpose back to token-major) reuses the
    freed O tiles; bias rides in yb; o2 = pj * rinv is a broadcast-AP
    tensor_tensor per jc; the o2+yb add runs on the otherwise-idle
    GPSIMD mid-stream (SBUF-only operands) and on the DVE for the
    latency-critical last pair.
  - O accumulation runs O_DELAY chunks behind exp so the static in-order
    PE program never stalls on exp or on the PSUM tag reuse WAR.
"""

import math
import os

import numpy as np
import ml_dtypes

import concourse.bass as bass
import concourse.bacc as bacc
import concourse.tile as tile
from concourse import mybir
from concourse.bass import broadcast_tensor_aps
from concourse import bass_utils as _bass_utils
from concourse.bass_utils import run_bass_kernel_spmd
from concourse._compat import get_trn_type

# NOTE: walrus's LDWEIGHTS dedup pass (--enable-ldw-opt=true) would remove
# the redundant weight loads between same-lhsT jc pairs, but walrus_driver
# crashes with it enabled (verified) -- that is why bass pins it off.

B, H, N, D = 8, 8, 1024, 64
SCALE = (512 // 8) ** (-0.5)  # 0.125
EPS = 1e-6
NCORES = 8
NB = N // 128
NW = N * NB
F32 = mybir.dt.float32
BF16 = mybir.dt.bfloat16
FP8 = mybir.dt.float8e4
U8 = mybir.dt.uint8
AX = mybir.AxisListType
OP = mybir.AluOpType
ACT = mybir.ActivationFunctionType
PM = mybir.MatmulPerfMode

A8 = 8.0 / math.log(2.0)
B8 = 7.0 * 8  # e4m3 Schraudolph magic (softmax cancels the offset choice)
DVE_CHUNKS = int(os.environ.get("KERNEL_DVE_CHUNKS", "7"))  # head-B chunks 0..DVE_CHUNKS-1 on the DVE, rest on ACT
O_DELAY = 3
WARMUP_MMS = int(os.environ.get("KERNEL_WARMUP_MMS", "0"))
YAP = 80  # padded ya8 row length (DoubleRow needs 16B-aligned Ko step)

LAST_RESULTS = None


def _emit(ctx, tc):
    nc = tc.nc
    qt = nc.dram_tensor("qt", [H // 2, 128, N], BF16, kind="ExternalInput")
    ya8 = nc.dram_tensor("ya8", [H, N, YAP], FP8, kind="ExternalInput")
    ybb = nc.dram_tensor("ybb", [H, N, D], BF16, kind="ExternalInput")
    wt = nc.dram_tensor("wt", [D, D], BF16, kind="ExternalInput")
    out = nc.dram_tensor("out", [H, N, D], BF16, kind="ExternalOutput")
    rscr = nc.dram_tensor("rscr", [2, 2, N], BF16)

    singles = ctx.enter_context(tc.tile_pool(name="singles", bufs=1))
    io = ctx.enter_context(tc.tile_pool(name="io", bufs=2))
    st = ctx.enter_context(tc.tile_pool(name="st", bufs=2))
    work = ctx.enter_context(tc.tile_pool(name="work", bufs=2))
    epool = ctx.enter_context(tc.tile_pool(name="epool", bufs=2))
    qpool = ctx.enter_context(tc.tile_pool(name="qpool", bufs=1))
    # PSUM: 4 banks of per-(head,jc) [128,512] S tiles + 4 banks of
    # per-(head,jc) O accumulators = all 8 banks.  S tiles are 1-bank and
    # exp consumes them per-half, so the WAR for chunk i+1's S matmul
    # releases ~a full chunk earlier than the old whole-chunk scheme.
    ps_s = ctx.enter_context(tc.tile_pool(name="ps_s", bufs=1, space="PSUM"))
    ps_o = ctx.enter_context(tc.tile_pool(name="ps_o", bufs=1, space="PSUM"))

    # HAM pre-warm: dense F=512 matmuls on garbage SBUF data (no DMA dep,
    # so the burst starts at t~0 and spans the ~8us DMA-ring ramp; the PE
    # must look continuously busy for a full 4096-cycle window to reach
    # K=8/8 before the first real S matmul)
    if WARMUP_MMS:
        warm_sb = singles.tile([64, 512], BF16)
        nc.vector.memset(warm_sb, 1.0)
        wps = ps_s.tile([128, 512], F32, tag="psSB0", name="warm")
        for k in range(WARMUP_MMS):
            nc.tensor.matmul(
                wps[0:64, 0:512], lhsT=warm_sb[:, 0:64], rhs=warm_sb,
                start=True, stop=True, tile_position=(0, 0),
                skip_group_check=True,
            )

    qT = [None] * (H // 2)

    def load_qt(p):
        # two half-loads: the first S matmuls (jc0) only need cols 0-511,
        # so pair 0's stream starts one DMA earlier
        q = qpool.tile([128, N], BF16, tag=f"qT{p}", name=f"qT{p}")
        nc.sync.dma_start(out=q[:, 0:512], in_=qt[p][:, 0:512])
        nc.sync.dma_start(out=q[:, 512:1024], in_=qt[p][:, 512:1024])
        qT[p] = q

    # qt pair 0 is the first thing the PE needs; wt is only read by the
    # first pair's tail proj, so its load can trail the qt halves
    load_qt(0)
    wt_sb = singles.tile([D, D], BF16)
    nc.sync.dma_start(out=wt_sb, in_=wt[:, :])
    load_qt(1)

    def bscale(dst, src, sc):
        sc3 = sc.rearrange("p (b u) -> p b u", u=1)
        sc_b, src_b = broadcast_tensor_aps(sc3, src)
        nc.vector.tensor_tensor(dst, src_b, sc_b, OP.mult)

    def make_state(p):
        hA, hB = 2 * p, 2 * p + 1
        yaA = io.tile([128, NB, YAP], FP8, tag="yaA")
        yaB = io.tile([128, NB, YAP], FP8, tag="yaB")
        ybA = io.tile([128, NB, D], BF16, tag="ybA")
        ybB = io.tile([128, NB, D], BF16, tag="ybB")
        nc.sync.dma_start(out=yaA, in_=ya8[hA].rearrange("(b p) d -> p b d", p=128))
        nc.sync.dma_start(out=yaB, in_=ya8[hB].rearrange("(b p) d -> p b d", p=128))
        EA = epool.tile([128, NW], FP8, tag="EA")
        EB = epool.tile([128, NW], FP8, tag="EB")
        return {
            "p": p, "q": qT[p], "hA": hA, "hB": hB,
            "heads": ((64, EB, yaB, ybB, "B"), (0, EA, yaA, ybA, "A")),
            "okptr": [0, 0], "odone": [0, 0], "otile": [None, None],
        }

    def emit_loads(P):
        # yb is only read in the epilogue, so its loads are issued AFTER
        # the previous pair's latency-critical denominator-bounce DMAs
        hA, hB = P["hA"], P["hB"]
        (_, _, _, ybB, _), (_, _, _, ybA, _) = P["heads"]
        nc.sync.dma_start(out=ybA, in_=ybb[hA].rearrange("(b p) d -> p b d", p=128))
        nc.sync.dma_start(out=ybB, in_=ybb[hB].rearrange("(b p) d -> p b d", p=128))

    def emit_o(P, hidx, flush=False):
        base, E, ytile, ybt, hc = P["heads"][hidx]
        E3 = E.rearrange("p (i n) -> p i n", n=N)
        while P["okptr"][hidx] < NB // 2:
            k = P["okptr"][hidx]
            # NOTE: emitting pair 0's O k-groups one chunk earlier (delay=2)
            # to fill its ramp measured WORSE in-trace: O-k0 waits on the
            # yaB input DMA (~14us, late in the DMA ramp) and head-of-line
            # blocks chunks 2-3's S matmuls in the in-order PE queue
            # (chunk-1->2 delta grew 3.8->5.0us).  delay=3 stands.
            if not flush and 2 * k + 3 > P["odone"][hidx]:
                return
            if k == 0:
                P["otile"][hidx] = [
                    ps_o.tile(
                        [128, 512], F32, tag=f"o{hc}{jc}",
                        name=f"ot{hc}{jc}{P['p']}",
                    )
                    for jc in range(2)
                ]
            for jc in range(2):
                nc.tensor.matmul(
                    P["otile"][hidx][jc][0 : D + 1, :],
                    lhsT=ytile[:, 2 * k : 2 * k + 2, 0 : D + 1],
                    rhs=E3[:, 2 * k : 2 * k + 2, jc * 512 : (jc + 1) * 512],
                    start=(k == 0), stop=(k == NB // 2 - 1),
                    perf_mode=PM.DoubleRow, tile_position=(0, 0),
                )
            P["okptr"][hidx] += 1

    def emit_chunk(P, i):
        q = P["q"]
        # head B: two 1-bank tiles, exp'd per-half on the DVE (cheap extra
        # instr, and the WAR for chunk i+1's B matmul releases at half-chunk
        # granularity).  head A: one 2-bank tile, exp'd whole on ACT (the
        # per-instruction ACTIVATE overhead is ~300 cycles, so halving A's
        # exps would cost ~2.9us/pair on the pacing engine).
        pssB = [
            ps_s.tile([128, 512], F32, tag=f"psSB{jc}", name=f"psSB{jc}")
            for jc in range(2)
        ]
        pssA = ps_s.tile([128, N], F32, tag="psSA", name="psSA")
        for jc in range(2):
            for hidx, (base, E, ytile, ybt, hc) in enumerate(P["heads"]):
                nc.tensor.matmul(
                    pssB[jc] if hc == "B"
                    else pssA[:, jc * 512 : (jc + 1) * 512],
                    lhsT=q[base : base + 64, i * 128 : (i + 1) * 128],
                    rhs=q[base : base + 64, jc * 512 : (jc + 1) * 512],
                    start=True, stop=True, tile_position=(base, 0),
                )
        for hidx, (base, E, ytile, ybt, hc) in enumerate(P["heads"]):
            # B's last chunk also on the DVE so both engines finish the
            # stream in parallel and the tail is not gated by ACT doing
            # both heads' final chunks back-to-back
            on_dve = hc == "B" and (i < DVE_CHUNKS - 1 or i == NB - 1)
            if hc == "B" and on_dve:
                for jc in range(2):
                    d0 = i * N + jc * 512
                    nc.vector.tensor_scalar(
                        out=E.bitcast(U8)[:, d0 : d0 + 512],
                        in0=pssB[jc], scalar1=A8, scalar2=B8,
                        op0=OP.mult, op1=OP.add,
                    )
            elif hc == "B":
                for jc in range(2):
                    d0 = i * N + jc * 512
                    nc.scalar.activation(
                        out=E[:, d0 : d0 + 512], in_=pssB[jc], func=ACT.Exp
                    )
            else:
                nc.scalar.activation(
                    out=E[:, i * N : (i + 1) * N], in_=pssA, func=ACT.Exp
                )
            P["odone"][hidx] += 1
        for hidx in range(2):
            emit_o(P, hidx)

    def emit_tail1(P, last=False):
        """O flush, OT evacuation, denominator bounce, proj.

        Final pair: head A is flushed FIRST (it was the long pole: second
        position + full-evac-gated bounce cost ~5us of exposed DMA latency
        at the very end), both heads evac their denominator row before the
        bulk so the bounce launches right at O-stop, and each head's bounce
        rides its own DMA ring (A=sync, B=scalar) so the two round-trips
        overlap completely."""
        p = P["p"]
        P["rT"] = st.tile([128, 2, NB], BF16, tag="rT", name="rT")
        order = (1, 0) if last else (0, 1)
        for hidx in order:
            base, E, ytile, ybt, hc = P["heads"][hidx]
            emit_o(P, hidx, flush=True)
            OT = work.tile([D + 1, N], BF16, tag=f"OT{hc}")
            if last:
                nc.vector.tensor_copy(
                    OT[D : D + 1, 0:512], P["otile"][hidx][0][D : D + 1, :]
                )
                nc.scalar.copy(
                    OT[D : D + 1, 512:1024], P["otile"][hidx][1][D : D + 1, :]
                )
                deng = nc.sync if hc == "A" else nc.scalar
                deng.dma_start(out=rscr[p % 2, hidx], in_=OT[D : D + 1, :])
                deng.dma_start(
                    out=P["rT"][:, hidx, :],
                    in_=rscr[p % 2, hidx].rearrange("(b p) -> p b", p=128),
                )
                nc.vector.tensor_copy(
                    OT[0:D, 0:512], P["otile"][hidx][0][0:D, :]
                )
                nc.scalar.copy(
                    OT[0:D, 512:1024], P["otile"][hidx][1][0:D, :]
                )
            else:
                # per-jc evac: DVE takes jc0, ACT takes jc1 (moving A-jc0 to
                # ACT to balance engine totals measured WORSE on hardware --
                # the serialized A evacs delay proj-A and the otile-ring WAR
                # more than the DVE relief is worth)
                nc.vector.tensor_copy(
                    OT[:, 0:512], P["otile"][hidx][0][0 : D + 1, :]
                )
                nc.scalar.copy(
                    OT[:, 512:1024], P["otile"][hidx][1][0 : D + 1, :]
                )
                nc.sync.dma_start(out=rscr[p % 2, hidx], in_=OT[D : D + 1, :])
                nc.sync.dma_start(
                    out=P["rT"][:, hidx, :],
                    in_=rscr[p % 2, hidx].rearrange("(b p) -> p b", p=128),
                )
            pj = [
                ps_o.tile(
                    [128, 512], F32, tag=f"o{hc}{jc}", name=f"pj{hc}{jc}{p}"
                )
                for jc in range(2)
            ]
            for b in range(NB):
                nc.tensor.matmul(
                    pj[b // 4][:, (b % 4) * 128 : (b % 4) * 128 + D],
                    lhsT=OT[0:D, b * 128 : (b + 1) * 128],
                    rhs=wt_sb,
                    start=True, stop=True, tile_position=(0, 0),
                )
            P["otile"][hidx] = pj

    def emit_tail2(P, last=False):
        """1/r scale, +yb, store."""
        order = (1, 0) if last else (0, 1)
        for hidx in order:
            base, E, ytile, ybt, hc = P["heads"][hidx]
            ho = P["hB"] if hc == "B" else P["hA"]
            rinv = st.tile([128, NB], F32, tag=f"rinv{hc}")
            nc.vector.reciprocal(rinv, P["rT"][:, hidx, :])
            o2 = work.tile([128, NB, D], BF16, tag=f"o2{hc}", name=f"o2{hc}")
            for jc in range(2):
                pj3 = P["otile"][hidx][jc].rearrange(
                    "p (b c) -> p b c", b=NB // 2
                )[:, :, 0:D]
                bscale(
                    o2[:, 4 * jc : 4 * jc + 4], pj3,
                    rinv[:, 4 * jc : 4 * jc + 4],
                )
            fin = work.tile([128, NB, D], BF16, tag=f"fin{hc}", name=f"fin{hc}")
            # the very last store rides the otherwise-done scalar ring so
            # the two final stores drain in parallel (ACT has no work left,
            # so the queue-DMA-blocks-ACTIVATE hazard doesn't apply here)
            # final pair: each head's store follows its own bounce ring
            # (A=sync, B=scalar) so the two stores drain in parallel
            eng = nc.scalar if last and hc == "B" else nc.sync
            o3 = out[ho].rearrange("(b p) d -> p b d", p=128)
            # fins stay on the DVE: any gpsimd use adds an SWDGE drain to
            # the NEFF postamble (the original design avoided gpsimd for
            # exactly this), and the DVE is not the pacer in the
            # HAM-cold regime anyway
            nc.vector.tensor_tensor(fin, o2, ybt, OP.add)
            eng.dma_start(out=o3, in_=fin)

    # software-pipelined pair schedule: each pair's first two chunk groups
    # are emitted around the previous pair's tail, so ACT/DVE stay fed with
    # exp work while the PE runs the tail's O-flush and proj matmuls and
    # the denominator-bounce DMA latency hides behind the second chunk.
    prev = None
    for p in range(H // 2):
        P = make_state(p)
        if prev is None:
            emit_loads(P)
        emit_chunk(P, 0)
        if prev is not None:
            emit_tail1(prev)
            emit_loads(P)
        emit_chunk(P, 1)
        if prev is not None:
            emit_tail2(prev)
        for i in range(2, NB):
            emit_chunk(P, i)
            if i == 2 and p + 2 < H // 2:
                load_qt(p + 2)
        prev = P
    emit_tail1(prev, last=True)
    emit_tail2(prev, last=True)


def build_program() -> bass.Bass:
    from contextlib import ExitStack

    nc = bacc.Bacc(get_trn_type() or "TRN2", target_bir_lowering=False)
    with tile.TileContext(nc) as tc:
        with ExitStack() as ctx:
            _emit(ctx, tc)
    nc.compile()
    return nc


def kernel(x, y, proj_w, proj_b, attn_gamma, sum_gamma0, sum_gamma1):
    global LAST_RESULTS
    x = np.asarray(x, dtype=np.float32)
    y = np.asarray(y, dtype=np.float32)
    proj_w = np.asarray(proj_w, dtype=np.float32)
    proj_b = np.asarray(proj_b, dtype=np.float32)
    g0 = math.exp(float(np.asarray(sum_gamma0)))
    g1 = math.exp(float(np.asarray(sum_gamma1)))
    w0 = g0 / (g0 + g1)
    w1 = g1 / (g0 + g1)
    c2 = 1.0 / (SCALE * float(np.asarray(attn_gamma)))

    nc = build_program()

    # q = l2norm(x) * sqrt(c2), transposed per pair: [B, 4, 128, N] with
    # head 2p on partitions 0-63 and head 2p+1 on partitions 64-127.
    q = (x * math.sqrt(c2) / np.sqrt((x * x).sum(-1, keepdims=True) + EPS))
    qt = np.ascontiguousarray(
        q.reshape(B, H // 2, 2, N, D).transpose(0, 1, 2, 4, 3).reshape(
            B, H // 2, 128, N
        )
    ).astype(ml_dtypes.bfloat16)
    # fp8 [y | 1 | pad] for the DoubleRow O matmuls; bf16 w0*y + w1*bias
    ya8 = np.zeros(y.shape[:-1] + (YAP,), ml_dtypes.float8_e4m3)
    ya8[..., 0:D] = y.astype(ml_dtypes.float8_e4m3)
    ya8[..., D] = 1.0
    ybb = (w0 * y + w1 * proj_b).astype(ml_dtypes.bfloat16)
    wt = (proj_w.T * w1).astype(ml_dtypes.bfloat16)

    in_maps = [
        {"qt": qt[c], "ya8": ya8[c], "ybb": ybb[c], "wt": wt}
        for c in range(NCORES)
    ]
    res = run_bass_kernel_spmd(nc, in_maps, list(range(NCORES)))
    LAST_RESULTS = res
    return np.stack(
        [res.results[c]["out"].astype(np.float32) for c in range(NCORES)], axis=0
    )

